# revision 59
# baseline (speedup 1.0000x reference)
"""Multi-head attention (B=4, S=2048, MODEL_DIM=2048, 16 heads, head dim 128)
on 8 Trainium2 NeuronCores.

Sharding: tensor-parallel over heads — 2 heads per core.  Each core projects
all 8192 tokens through its 256-column slice of W_Q/W_K/W_V, runs attention
for its heads, applies its 256-row slice of W_O, and a per-batch
ReduceScatter sums the partial outputs (overlapping compute; each core keeps
its 1/8 row-chunk and the host reassembles the full output).  The last
batch's reductions fire per row-chunk as q-blocks finish so only the final
small chunk is exposed at the tail.

Numerics: the softmax path is precision-critical (scores have std ~2048, so
the softmax is near-argmax; small score errors flip near-tie rows).  The Q/K
projections and the Q.K^T scores run as a 2-pass scheme: one fp16 hi*hi
matmul pass (exact products, f32 accum) plus ONE fp8 DoubleRow pass that
computes both cross terms (hi*lo + lo*hi) at 2 MACs/cell/cycle.  Hi/lo limbs
carry power-of-2 scales chosen so the fp16 pass and the fp8 pass accumulate
into the same PSUM bank at a common scale (projection psum = 2^12 * Q,
score psum = 2^9 * S), so no extra combine ops are needed.  The value path
runs single-pass fp16 (V projection, P.V, W_O) with exact f32 softmax
statistics throughout.
"""

import os
import sys
import types

sys.path.insert(0, "/opt/trn_rl_repo")

import numpy as np
import ml_dtypes

# ─────────────────────────────── constants ───────────────────────────────
B, S, D = 4, 2048, 2048
H, R = 16, 128
N_CORES = 8
HPC = H // N_CORES          # heads per core = 2
RW = HPC * R                # per-core projection width = 256
T = B * S                   # 8192 tokens
DC = D // 128               # 16 contraction chunks
SCALE = 1.0 / (R ** 0.5)
SCALE9 = SCALE / 512.0      # score psum carries a 2^9 scale

F8NP = ml_dtypes.float8_e4m3   # TRN float8e4 (max +-240)

XH_BUFS = int(os.environ.get("K_XH_BUFS", "28"))
X8_BUFS = int(os.environ.get("K_X8_BUFS", "20"))

# last-batch reduce-scatter chunks: (start row in batch, nrows), fired as
# soon as the covering q-blocks finish; the final small chunk minimizes the
# exposed tail
TAIL_CHUNKS = [(0, 1536), (1536, 512)]

LAST_EXEC_TIME_NS = [None]
LAST_RESULTS = [None]


# ───────────────────────── harness glue (inlined) ─────────────────────────
def _install_ntff_hook():
    """Wire the missing antenv.axon_hooks module so trace=True can profile."""
    try:
        import antenv.axon_hooks  # noqa: F401
        return
    except ImportError:
        pass
    try:
        import antenv
        from trn_agent_boot.trn_boot import _ntff_profile_via_ctypes
    except ImportError:
        return
    mod = types.ModuleType("antenv.axon_hooks")
    _hook = [None]
    mod.set_axon_ntff_profile_hook = lambda h: _hook.__setitem__(0, h)
    mod.get_axon_ntff_profile_hook = lambda: _hook[0]
    antenv.axon_hooks = mod
    sys.modules["antenv.axon_hooks"] = mod
    try:
        mod.set_axon_ntff_profile_hook(
            _ntff_profile_via_ctypes("/opt/axon/libaxon_pjrt.so")
        )
    except Exception:
        pass


def _split_excess_waits(nc, max_waits=1):
    """walrus on this toolchain rejects >1 sem-wait per instruction; hoist
    the excess onto preceding same-engine NoOps."""
    from concourse import mybir

    for fn in nc.m.functions:
        for bb in fn.blocks:
            insts = list(bb.instructions)
            out = []
            changed = False
            for inst in insts:
                si = inst.sync_info
                if si is not None and si.on_wait and len(si.on_wait) > max_waits:
                    waits = list(si.on_wait)
                    chunks = [
                        waits[i : i + max_waits]
                        for i in range(0, len(waits), max_waits)
                    ]
                    for ci, chunk in enumerate(chunks[:-1]):
                        out.append(
                            mybir.InstNoOp(
                                name=f"{inst.name}-ws{ci}",
                                engine=inst.engine,
                                ins=[],
                                outs=[],
                                sync_info=mybir.SyncInfo(
                                    on_wait=list(chunk), on_update=[]
                                ),
                                text_hint="waitsplit",
                            )
                        )
                    si.on_wait = list(chunks[-1])
                    changed = True
                out.append(inst)
            if changed:
                try:
                    bb.instructions = out
                except Exception:
                    bb.instructions.clear()
                    for i in out:
                        bb.instructions.append(i)


# ───────────────────────────── device kernel ─────────────────────────────
def _build_nc():
    from contextlib import ExitStack

    import concourse.bass as bass
    import concourse.tile as tile
    from concourse import mybir
    from concourse.masks import make_identity

    f32 = mybir.dt.float32
    f16 = mybir.dt.float16
    bf16 = mybir.dt.bfloat16
    f8 = mybir.dt.float8e4
    AX = mybir.AxisListType
    EXP = mybir.ActivationFunctionType.Exp
    DR = mybir.MatmulPerfMode.DoubleRow
    MUL = mybir.AluOpType.mult
    SUB = mybir.AluOpType.subtract
    MAX = mybir.AluOpType.max

    nc = bass.Bass(
        "TRN2", target_bir_lowering=False, debug=False, num_devices=N_CORES
    )

    xh6_ap = nc.dram_tensor("xh6", [D, T], f16, kind="ExternalInput").ap()
    xl8_ap = nc.dram_tensor("xl8", [D, T], f8, kind="ExternalInput").ap()
    w6_ap = {
        m: nc.dram_tensor(f"w{m}6", [D, RW], f16, kind="ExternalInput").ap()
        for m in ("q", "k")
    }
    w8_ap = {
        m: nc.dram_tensor(f"w{m}8", [D, 2 * RW], f8, kind="ExternalInput").ap()
        for m in ("q", "k")
    }
    wv_ap = nc.dram_tensor("wv6m", [D, RW], f16, kind="ExternalInput").ap()
    wo_ap = nc.dram_tensor("wo", [RW, R], f16, kind="ExternalInput").ap()
    # ReduceScatter output: each core keeps its contiguous 1/8 of every
    # reduced row-chunk; the host reassembles the full [T, R].
    out_ap = nc.dram_tensor(
        "out", [B * (S // N_CORES), R], f32, kind="ExternalOutput"
    ).ap()
    ar_in = nc.dram_tensor("ar_in", [T, R], f32)
    rs_out = nc.dram_tensor("rs_out", [B * (S // N_CORES), R], f32)

    with tile.TileContext(nc) as tc, ExitStack() as ctx:
        P = lambda **kw: ctx.enter_context(tc.tile_pool(**kw))
        const = P(name="const", bufs=1)
        x_pool = P(name="x", bufs=XH_BUFS)
        qkv_pool = P(name="qkv", bufs=2)
        lo_pool = P(name="lo", bufs=2)
        s_pool = P(name="s", bufs=2)
        p_pool = P(name="p", bufs=2)
        pt_pool = P(name="pt", bufs=2)
        ot_pool = P(name="ot", bufs=3)
        tmp_pool = P(name="tmp", bufs=2)
        stats = P(name="stats", bufs=4)
        ps = P(name="ps", bufs=1, space="PSUM")  # bufs set per tile() call

        def load_x_tg(t0):
            """Create + DMA the X tiles for one 512-token group."""
            xh_t, x8_t = [], []
            # all xh6 DMAs first: the fp16 main chains and V are the first
            # consumers, the fp8 pair tiles are only needed by the later DR
            # chains, so don't let xl8 transfers queue ahead of xh6
            for dc in range(DC):
                th = x_pool.tile([128, 512], f16, tag="xh", name="xh_t")
                nc.sync.dma_start(
                    th[:], xh6_ap[dc * 128 : (dc + 1) * 128, t0 : t0 + 512]
                )
                xh_t.append(th)
            for dc in range(DC):
                t8 = x_pool.tile([128, 2, 2, 256], f8, tag="x8", name="x8_t", bufs=X8_BUFS)
                # slot0 (xh8) derived on-device from xh6 — saves HBM traffic
                nc.vector.tensor_scalar_mul(t8[:, 0, :, :], xh_t[dc][:], 2.0 ** -6)
                nc.sync.dma_start(
                    t8[:, 1, :, :],
                    xl8_ap[dc * 128 : (dc + 1) * 128, t0 : t0 + 512],
                )
                x8_t.append(t8)
            return xh_t, x8_t

        # resident weights
        # fp16 mains: [128, DC*RW], column block dc holds W[dc*128:(dc+1)*128, :]
        # (wq6 first, then batch-0 X prefetch, then the rest — so the first
        # projection chain's operands land early and the PE starts ~15us in)
        w6_sb = {
            m: const.tile([128, DC * RW], f16, tag=f"w{m}6", name=f"w{m}6_sb")
            for m in ("q", "k")
        }
        # fp8 pair weights: [128, 2slot, DC, HPC, 128]; slot0=wl*2^12, slot1=wh
        w8_sb = {
            m: const.tile(
                [128, 2, DC, HPC, 128], f8, tag=f"w{m}8", name=f"w{m}8_sb"
            )
            for m in ("q", "k")
        }

        def load_w(m):
            for dc in range(DC):
                nc.sync.dma_start(
                    w6_sb[m][:, dc * RW : (dc + 1) * RW],
                    w6_ap[m][dc * 128 : (dc + 1) * 128, :],
                )
            for dc in range(DC):
                nc.sync.dma_start(
                    w8_sb[m][:, :, dc, :, :],
                    w8_ap[m][dc * 128 : (dc + 1) * 128, :],
                )

        # DMA issue order follows first use: V chains run first (wv + tg0 X),
        # then K chains, then Q chains, then the next token group
        wv_sb = const.tile([128, DC * RW], f16, tag="wv", name="wv_sb")
        for dc in range(DC):
            nc.sync.dma_start(
                wv_sb[:, dc * RW : (dc + 1) * RW],
                wv_ap[dc * 128 : (dc + 1) * 128, :],
            )
        pre_x = {0: load_x_tg(0)}
        load_w("k")
        load_w("q")
        pre_x[1] = load_x_tg(512)
        wo_sb = const.tile([128, HPC * R], f16, tag="wo", name="wo_sb")
        for rh in range(HPC):
            nc.sync.dma_start(
                wo_sb[:, rh * R : (rh + 1) * R],
                wo_ap[rh * 128 : (rh + 1) * 128, :],
            )
        ident = const.tile([128, 128], f16, tag="ident", name="ident")
        make_identity(nc, ident[:])

        a_state = {}

        def gen_phase_a(b):
            """Projections for batch b, yielding after each matmul chain (32
            yields) so the caller can interleave them with the previous
            batch's attention iterations."""
            tb0 = b * S
            # hi fp16 tiles: Q^T at 2^4, K^T at 2^5 — [128 rank, S tokens]
            hi = {
                (m, rh): qkv_pool.tile(
                    [128, S], f16, tag=f"{m}h{rh}", name=f"{m}h{rh}"
                )
                for m in ("q", "k")
                for rh in range(HPC)
            }
            # fp8 pair tiles:
            #  q: [128, 2slot, 16qb, 128]  slot0=qh*2^-1, slot1=ql*2^10
            #  k: [128, 2slot, 8chunk, 256] slot0=kl*2^10, slot1=kh*2^-1
            q8t = {
                rh: qkv_pool.tile(
                    [128, 2, 16, 128], f8, tag=f"q8{rh}", name=f"q8{rh}"
                )
                for rh in range(HPC)
            }
            k8t = {
                rh: qkv_pool.tile(
                    [128, 2, 8, 256], f8, tag=f"k8{rh}", name=f"k8{rh}"
                )
                for rh in range(HPC)
            }
            v_sb = qkv_pool.tile([128, DC * RW], f16, tag="v", name="v_sb")
            a_state[b] = (hi, q8t, k8t, v_sb)

            for tg in range(4):
                t0 = tb0 + tg * 512
                if b == 0 and tg in pre_x:
                    xh_t, x8_t = pre_x.pop(tg)
                else:
                    xh_t, x8_t = load_x_tg(t0)

                # Q^T, K^T: fp16 hi*hi chain + fp8 DoubleRow cross chain
                for m in ("k",):
                    for rh in range(HPC):
                        psp = ps.tile([128, 512], f32, tag="pa", bufs=2, name="ps_proj")
                        for dc in range(DC):
                            nc.tensor.matmul(
                                psp[:],
                                lhsT=w6_sb[m][
                                    :, dc * RW + rh * 128 : dc * RW + rh * 128 + 128
                                ],
                                rhs=xh_t[dc][:],
                                start=(dc == 0),
                                stop=False,
                            )
                        for dc in range(DC):
                            nc.tensor.matmul(
                                psp[:],
                                lhsT=w8_sb[m][:, :, dc, rh, :],
                                rhs=x8_t[dc][:, :, :, :],
                                perf_mode=DR,
                                start=False,
                                stop=(dc == DC - 1),
                            )
                        # drain: hi fp16 copy, lo via scalar_tensor_tensor,
                        # then two fp8 converts into the pair tiles
                        hs = hi[(m, rh)][:, tg * 512 : (tg + 1) * 512]
                        sc = 2.0 ** -8 if m == "q" else 2.0 ** -7
                        nc.scalar.mul(hs, psp[:], sc)
                        lo = lo_pool.tile([128, 512], f16, tag="lo", name="lo")
                        nc.vector.scalar_tensor_tensor(
                            lo[:], psp[:], sc, hs, MUL, SUB
                        )
                        if m == "q":
                            nc.scalar.mul(
                                q8t[rh][:, 0, tg * 4 : (tg + 1) * 4, :],
                                hs, 2.0 ** -5,
                            )
                            nc.scalar.mul(
                                q8t[rh][:, 1, tg * 4 : (tg + 1) * 4, :],
                                lo[:], 2.0 ** 6,
                            )
                        else:
                            nc.scalar.mul(
                                k8t[rh][:, 1, tg * 2 : (tg + 1) * 2, :],
                                hs, 2.0 ** -6,
                            )
                            nc.scalar.mul(
                                k8t[rh][:, 0, tg * 2 : (tg + 1) * 2, :],
                                lo[:], 2.0 ** 5,
                            )
                        yield

                # V after K: each V matmul needs every xh tile of the
                # group, while the K mains consume them sequentially and
                # can ramp with the arriving DMAs; Q still runs last
                for tb in range(4):
                    psv = ps.tile([128, RW], f32, tag="pa", bufs=2, name="ps_vproj")
                    for dc in range(DC):
                        nc.tensor.matmul(
                            psv[:],
                            lhsT=xh_t[dc][:, tb * 128 : (tb + 1) * 128],
                            rhs=wv_sb[:, dc * RW : (dc + 1) * RW],
                            start=(dc == 0),
                            stop=(dc == DC - 1),
                        )
                    tbi = tg * 4 + tb
                    nc.scalar.copy(v_sb[:, tbi * RW : (tbi + 1) * RW], psv[:])
                    yield

                # Q^T, K^T: fp16 hi*hi chain + fp8 DoubleRow cross chain
                for m in ("q",):
                    for rh in range(HPC):
                        psp = ps.tile([128, 512], f32, tag="pa", bufs=2, name="ps_proj")
                        for dc in range(DC):
                            nc.tensor.matmul(
                                psp[:],
                                lhsT=w6_sb[m][
                                    :, dc * RW + rh * 128 : dc * RW + rh * 128 + 128
                                ],
                                rhs=xh_t[dc][:],
                                start=(dc == 0),
                                stop=False,
                            )
                        for dc in range(DC):
                            nc.tensor.matmul(
                                psp[:],
                                lhsT=w8_sb[m][:, :, dc, rh, :],
                                rhs=x8_t[dc][:, :, :, :],
                                perf_mode=DR,
                                start=False,
                                stop=(dc == DC - 1),
                            )
                        # drain: hi fp16 copy, lo via scalar_tensor_tensor,
                        # then two fp8 converts into the pair tiles
                        hs = hi[(m, rh)][:, tg * 512 : (tg + 1) * 512]
                        sc = 2.0 ** -8 if m == "q" else 2.0 ** -7
                        nc.scalar.mul(hs, psp[:], sc)
                        lo = lo_pool.tile([128, 512], f16, tag="lo", name="lo")
                        nc.vector.scalar_tensor_tensor(
                            lo[:], psp[:], sc, hs, MUL, SUB
                        )
                        if m == "q":
                            nc.scalar.mul(
                                q8t[rh][:, 0, tg * 4 : (tg + 1) * 4, :],
                                hs, 2.0 ** -5,
                            )
                            nc.scalar.mul(
                                q8t[rh][:, 1, tg * 4 : (tg + 1) * 4, :],
                                lo[:], 2.0 ** 6,
                            )
                        else:
                            nc.scalar.mul(
                                k8t[rh][:, 1, tg * 2 : (tg + 1) * 2, :],
                                hs, 2.0 ** -6,
                            )
                            nc.scalar.mul(
                                k8t[rh][:, 0, tg * 2 : (tg + 1) * 2, :],
                                lo[:], 2.0 ** 5,
                            )
                        yield

        for _ in gen_phase_a(0):  # batch 0 projections up front
            pass

        for b in range(B):
            tb0 = b * S
            hi, q8t, k8t, v_sb = a_state.pop(b)
            nxt = gen_phase_a(b + 1) if b + 1 < B else iter(())

            # ── phase B: attention, heads interleaved per q-block, with the
            # next batch's projection chains interleaved one per iteration ──
            for qb in range(16):
                o2s = []
                for h in range(HPC):
                    q0 = qb * 128
                    # scores psum = 2^9 * S: fp16 (qh*2^4)·(kh*2^5) + fp8 DR
                    # (qh*2^-1)·(kl*2^10) + (ql*2^10)·(kh*2^-1)
                    s_sb = s_pool.tile([128, S], f32, tag="s", name="s_sb")
                    pmax = stats.tile([128, 4], f32, tag="pmax", name="pmax")
                    # kt pairs: both fp16 mains back-to-back (one qh4 LDW
                    # region), then 4 DR matmuls sharing the q8 stationary
                    for kp in range(2):
                        pss = [
                            ps.tile([128, 512], f32, tag="s", bufs=3, name="ps_s")
                            for _ in range(2)
                        ]
                        for i in range(2):
                            kt = kp * 2 + i
                            nc.tensor.matmul(
                                pss[i][:],
                                lhsT=hi[("q", h)][:, q0 : q0 + 128],
                                rhs=hi[("k", h)][:, kt * 512 : (kt + 1) * 512],
                                start=True,
                                stop=False,
                            )
                        for i in range(2):
                            kt = kp * 2 + i
                            nc.tensor.matmul(
                                pss[i][:],
                                lhsT=q8t[h][:, :, qb, :],
                                rhs=k8t[h][:, :, kt * 2 : kt * 2 + 2, :],
                                perf_mode=DR,
                                start=False,
                                stop=True,
                            )
                        for i in range(2):
                            kt = kp * 2 + i
                            sl = s_sb[:, kt * 512 : (kt + 1) * 512]
                            # balance Scalar vs Vector: mid-run the Scalar
                            # engine also carries projection drains, so one
                            # copy goes to the DVE; the last batch instead
                            # overloads the DVE (reductions + pt copies), so
                            # all copies stay on Scalar there
                            if kt == 3 and b < B - 1:
                                nc.vector.tensor_copy(sl, pss[i][:])
                            else:
                                nc.scalar.copy(sl, pss[i][:])
                            nc.vector.reduce_max(
                                pmax[:, kt : kt + 1], pss[i][:], axis=AX.X
                            )

                    negmax = stats.tile([128, 1], f32, tag="negmax", name="negmax")
                    nc.vector.reduce_max(negmax[:], pmax[:], axis=AX.X, negate=True)
                    bias = stats.tile([128, 1], f32, tag="bias", name="bias")
                    nc.vector.tensor_scalar_mul(bias[:], negmax[:], SCALE9)
                    p_t = p_pool.tile([128, S], f16, tag="p", name="p_t")
                    pt_sb = pt_pool.tile([128, DC * 128], f16, tag="pt", name="pt_sb")
                    ssum4 = stats.tile([128, 4], f32, tag="ssum4", name="ssum4")
                    # two 1024-wide exps (fewer Scalar dispatches); the
                    # transposes/psum copies still pipeline per 512 slice
                    for ep in range(2):
                        nc.scalar.activation(
                            p_t[:, ep * 1024 : (ep + 1) * 1024],
                            s_sb[:, ep * 1024 : (ep + 1) * 1024],
                            EXP, bias=bias[:], scale=SCALE9,
                            accum_out=ssum4[:, ep : ep + 1],
                        )
                        for kt in (ep * 2, ep * 2 + 1):
                            pst = ps.tile(
                                [128, 512], f16, tag="pt", bufs=2, name="ps_pt"
                            )
                            for j in range(4):
                                kc = kt * 4 + j
                                nc.tensor.transpose(
                                    pst[:, j * 128 : (j + 1) * 128],
                                    p_t[:, kc * 128 : (kc + 1) * 128],
                                    ident[:],
                                )
                            nc.vector.tensor_copy(
                                pt_sb[:, kt * 512 : (kt + 1) * 512], pst[:]
                            )
                    ssum = stats.tile([128, 1], f32, tag="ssum", name="ssum")
                    nc.vector.reduce_sum(ssum[:], ssum4[:, 0:2], axis=AX.X)
                    rc = stats.tile([128, 1], f32, tag=f"recip{h}", name="rc")
                    nc.vector.reciprocal(rc[:], ssum[:])

                    # attn = P @ V, accumulated transposed: O^T [128 r, 128 q]
                    ps_ot = ps.tile([128, 128], f32, tag="ot", bufs=1, name="ps_ot")
                    for kc in range(DC):
                        nc.tensor.matmul(
                            ps_ot[:],
                            lhsT=v_sb[
                                :, kc * RW + h * 128 : kc * RW + h * 128 + 128
                            ],
                            rhs=pt_sb[:, kc * 128 : (kc + 1) * 128],
                            start=(kc == 0),
                            stop=(kc == DC - 1),
                        )
                    ot_sb = ot_pool.tile([128, 128], f16, tag="ot", name="ot_sb")
                    if b == B - 1:
                        nc.vector.tensor_copy(ot_sb[:], ps_ot[:])
                    else:
                        nc.scalar.copy(ot_sb[:], ps_ot[:])

                    # out2 [128 q, 128] = O^T.T @ Wo_h  (fp16)
                    ps_o2 = ps.tile([128, 512], f32, tag="pa", bufs=2, name="ps_o2")
                    nc.tensor.matmul(
                        ps_o2[:, 0:128],
                        lhsT=ot_sb[:],
                        rhs=wo_sb[:, h * R : (h + 1) * R],
                        start=True,
                        stop=True,
                    )
                    tmp = tmp_pool.tile([128, 128], f32, tag=f"o2s{h}", name="tmp")
                    if b == B - 1:
                        nc.vector.tensor_scalar_mul(tmp[:], ps_o2[:, 0:128], rc[:])
                    else:
                        nc.scalar.mul(tmp[:], ps_o2[:, 0:128], rc[:])
                    o2s.append(tmp)
                    next(nxt, None)  # interleave one next-batch proj chain
                res = tmp_pool.tile([128, 128], f32, tag="res", name="res")
                nc.vector.tensor_add(res[:], o2s[0][:], o2s[1][:])
                nc.sync.dma_start(
                    ar_in.ap()[tb0 + qb * 128 : tb0 + (qb + 1) * 128, :],
                    res[:],
                )
                if b == B - 1:
                    done = (qb + 1) * 128
                    off = 0
                    for r0, nr in TAIL_CHUNKS:
                        if r0 + nr == done:
                            o0 = b * 256 + off
                            on = nr // N_CORES
                            nc.gpsimd.collective_compute(
                                "ReduceScatter",
                                mybir.AluOpType.add,
                                replica_groups=[list(range(N_CORES))],
                                ins=[ar_in.ap()[tb0 + r0 : tb0 + r0 + nr, :]],
                                outs=[rs_out.ap()[o0 : o0 + on, :]],
                            )
                            nc.sync.dma_start(
                                out_ap[o0 : o0 + on, :],
                                rs_out.ap()[o0 : o0 + on, :],
                            )
                        off += nr // N_CORES

            for _ in nxt:  # drain any leftover projection chains
                pass
            # reduce-scatter this batch's slice while the next batch computes
            if b < B - 1:
                nc.gpsimd.collective_compute(
                    "ReduceScatter",
                    mybir.AluOpType.add,
                    replica_groups=[list(range(N_CORES))],
                    ins=[ar_in.ap()[tb0 : tb0 + S, :]],
                    outs=[rs_out.ap()[b * 256 : (b + 1) * 256, :]],
                )
                nc.sync.dma_start(
                    out_ap[b * 256 : (b + 1) * 256, :],
                    rs_out.ap()[b * 256 : (b + 1) * 256, :],
                )

    return nc


# ─────────────────────────────── host entry ───────────────────────────────
def _f8(a):
    return np.asarray(a, np.float32).astype(F8NP)


def kernel(X, mask, W_Q, W_K, W_V, W_O):
    _install_ntff_hook()
    from concourse.bass_utils import run_bass_kernel_spmd

    X2 = np.ascontiguousarray(
        np.asarray(X, dtype=np.float32).reshape(T, D).T
    )  # [D, T]
    xh32 = X2.astype(np.float16).astype(np.float32)
    xl32 = X2 - xh32
    xh6 = (xh32 * 64.0).astype(np.float16)
    xl8 = _f8(xl32 * 4096.0)
    W_Q = np.asarray(W_Q, np.float32)
    W_K = np.asarray(W_K, np.float32)
    W_V = np.asarray(W_V, np.float32)
    W_O = np.asarray(W_O, np.float32)

    in_maps = []
    for c in range(N_CORES):
        cols = slice(c * RW, (c + 1) * RW)

        def wsplit(Wfull):
            wc = Wfull[:, cols]
            wh32 = wc.astype(np.float16).astype(np.float32)
            wl32 = wc - wh32
            w6 = (wh32 * 64.0).astype(np.float16)
            w8 = np.empty((D, 2, RW), F8NP)
            w8[:, 0, :] = _f8(wl32 * 4096.0)
            w8[:, 1, :] = _f8(wh32)
            return w6, w8.reshape(D, 2 * RW)

        wq6, wq8 = wsplit(W_Q)
        wk6, wk8 = wsplit(W_K)
        wv6m = (
            W_V[:, cols].astype(np.float16).astype(np.float32) * 2.0 ** -6
        ).astype(np.float16)
        in_maps.append(
            {
                "xh6": xh6,
                "xl8": xl8,
                "wq6": wq6,
                "wq8": wq8,
                "wk6": wk6,
                "wk8": wk8,
                "wv6m": wv6m,
                "wo": np.ascontiguousarray(W_O[cols, :]).astype(np.float16),
            }
        )

    nc = _build_nc()
    _split_excess_waits(nc)
    trace = bool(int(os.environ.get("KERNEL_TRACE", "0")))
    res = run_bass_kernel_spmd(
        nc, in_maps, list(range(N_CORES)), trace=trace
    )
    LAST_EXEC_TIME_NS[0] = res.exec_time_ns
    LAST_RESULTS[0] = res
    # reassemble the reduce-scattered output: batches 0..B-2 were scattered
    # as whole [S, R] chunks (core c holds rows c*256..), the last batch as
    # four 512-row chunks (core c holds rows j*512 + c*64..)
    full = np.empty((T, R), np.float32)
    for c in range(N_CORES):
        oc = np.asarray(res.results[c]["out"], dtype=np.float32)
        for b in range(B - 1):
            full[b * S + c * 256 : b * S + (c + 1) * 256] = oc[
                b * 256 : (b + 1) * 256
            ]
        b = B - 1
        off = 0
        for r0, nr in TAIL_CHUNKS:
            on = nr // N_CORES
            full[b * S + r0 + c * on : b * S + r0 + (c + 1) * on] = oc[
                b * 256 + off : b * 256 + off + on
            ]
            off += on
    return full.reshape(B, S, R)


# revision 60
# speedup vs baseline: 1.0017x; 1.0017x over previous
"""Multi-head attention (B=4, S=2048, MODEL_DIM=2048, 16 heads, head dim 128)
on 8 Trainium2 NeuronCores.

Sharding: tensor-parallel over heads — 2 heads per core.  Each core projects
all 8192 tokens through its 256-column slice of W_Q/W_K/W_V, runs attention
for its heads, applies its 256-row slice of W_O, and a per-batch
ReduceScatter sums the partial outputs (overlapping compute; each core keeps
its 1/8 row-chunk and the host reassembles the full output).  The last
batch's reductions fire per row-chunk as q-blocks finish so only the final
small chunk is exposed at the tail.

Numerics: the softmax path is precision-critical (scores have std ~2048, so
the softmax is near-argmax; small score errors flip near-tie rows).  The Q/K
projections and the Q.K^T scores run as a 2-pass scheme: one fp16 hi*hi
matmul pass (exact products, f32 accum) plus ONE fp8 DoubleRow pass that
computes both cross terms (hi*lo + lo*hi) at 2 MACs/cell/cycle.  Hi/lo limbs
carry power-of-2 scales chosen so the fp16 pass and the fp8 pass accumulate
into the same PSUM bank at a common scale (projection psum = 2^12 * Q,
score psum = 2^9 * S), so no extra combine ops are needed.  The value path
runs single-pass fp16 (V projection, P.V, W_O) with exact f32 softmax
statistics throughout.
"""

import os
import sys
import types

sys.path.insert(0, "/opt/trn_rl_repo")

import numpy as np
import ml_dtypes

# ─────────────────────────────── constants ───────────────────────────────
B, S, D = 4, 2048, 2048
H, R = 16, 128
N_CORES = 8
HPC = H // N_CORES          # heads per core = 2
RW = HPC * R                # per-core projection width = 256
T = B * S                   # 8192 tokens
DC = D // 128               # 16 contraction chunks
SCALE = 1.0 / (R ** 0.5)
SCALE9 = SCALE / 512.0      # score psum carries a 2^9 scale

F8NP = ml_dtypes.float8_e4m3   # TRN float8e4 (max +-240)

XH_BUFS = int(os.environ.get("K_XH_BUFS", "28"))
X8_BUFS = int(os.environ.get("K_X8_BUFS", "20"))

# last-batch reduce-scatter chunks: (start row in batch, nrows), fired as
# soon as the covering q-blocks finish; the final small chunk minimizes the
# exposed tail
TAIL_CHUNKS = [(0, 1536), (1536, 512)]

LAST_EXEC_TIME_NS = [None]
LAST_RESULTS = [None]


# ───────────────────────── harness glue (inlined) ─────────────────────────
def _install_ntff_hook():
    """Wire the missing antenv.axon_hooks module so trace=True can profile."""
    try:
        import antenv.axon_hooks  # noqa: F401
        return
    except ImportError:
        pass
    try:
        import antenv
        from trn_agent_boot.trn_boot import _ntff_profile_via_ctypes
    except ImportError:
        return
    mod = types.ModuleType("antenv.axon_hooks")
    _hook = [None]
    mod.set_axon_ntff_profile_hook = lambda h: _hook.__setitem__(0, h)
    mod.get_axon_ntff_profile_hook = lambda: _hook[0]
    antenv.axon_hooks = mod
    sys.modules["antenv.axon_hooks"] = mod
    try:
        mod.set_axon_ntff_profile_hook(
            _ntff_profile_via_ctypes("/opt/axon/libaxon_pjrt.so")
        )
    except Exception:
        pass


def _split_excess_waits(nc, max_waits=1):
    """walrus on this toolchain rejects >1 sem-wait per instruction; hoist
    the excess onto preceding same-engine NoOps."""
    from concourse import mybir

    for fn in nc.m.functions:
        for bb in fn.blocks:
            insts = list(bb.instructions)
            out = []
            changed = False
            for inst in insts:
                si = inst.sync_info
                if si is not None and si.on_wait and len(si.on_wait) > max_waits:
                    waits = list(si.on_wait)
                    chunks = [
                        waits[i : i + max_waits]
                        for i in range(0, len(waits), max_waits)
                    ]
                    for ci, chunk in enumerate(chunks[:-1]):
                        out.append(
                            mybir.InstNoOp(
                                name=f"{inst.name}-ws{ci}",
                                engine=inst.engine,
                                ins=[],
                                outs=[],
                                sync_info=mybir.SyncInfo(
                                    on_wait=list(chunk), on_update=[]
                                ),
                                text_hint="waitsplit",
                            )
                        )
                    si.on_wait = list(chunks[-1])
                    changed = True
                out.append(inst)
            if changed:
                try:
                    bb.instructions = out
                except Exception:
                    bb.instructions.clear()
                    for i in out:
                        bb.instructions.append(i)


# ───────────────────────────── device kernel ─────────────────────────────
def _build_nc():
    from contextlib import ExitStack

    import concourse.bass as bass
    import concourse.tile as tile
    from concourse import mybir
    from concourse.masks import make_identity

    f32 = mybir.dt.float32
    f16 = mybir.dt.float16
    bf16 = mybir.dt.bfloat16
    f8 = mybir.dt.float8e4
    AX = mybir.AxisListType
    EXP = mybir.ActivationFunctionType.Exp
    DR = mybir.MatmulPerfMode.DoubleRow
    MUL = mybir.AluOpType.mult
    SUB = mybir.AluOpType.subtract
    MAX = mybir.AluOpType.max

    nc = bass.Bass(
        "TRN2", target_bir_lowering=False, debug=False, num_devices=N_CORES
    )

    xh6_ap = nc.dram_tensor("xh6", [D, T], f16, kind="ExternalInput").ap()
    xl8_ap = nc.dram_tensor("xl8", [D, T], f8, kind="ExternalInput").ap()
    w6_ap = {
        m: nc.dram_tensor(f"w{m}6", [D, RW], f16, kind="ExternalInput").ap()
        for m in ("q", "k")
    }
    w8_ap = {
        m: nc.dram_tensor(f"w{m}8", [D, 2 * RW], f8, kind="ExternalInput").ap()
        for m in ("q", "k")
    }
    wv_ap = nc.dram_tensor("wv6m", [D, RW], f16, kind="ExternalInput").ap()
    wo_ap = nc.dram_tensor("wo", [RW, R], f16, kind="ExternalInput").ap()
    # ReduceScatter output: each core keeps its contiguous 1/8 of every
    # reduced row-chunk; the host reassembles the full [T, R].
    out_ap = nc.dram_tensor(
        "out", [B * (S // N_CORES), R], f32, kind="ExternalOutput"
    ).ap()
    ar_in = nc.dram_tensor("ar_in", [T, R], f32)
    rs_out = nc.dram_tensor("rs_out", [B * (S // N_CORES), R], f32)

    with tile.TileContext(nc) as tc, ExitStack() as ctx:
        P = lambda **kw: ctx.enter_context(tc.tile_pool(**kw))
        const = P(name="const", bufs=1)
        x_pool = P(name="x", bufs=XH_BUFS)
        qkv_pool = P(name="qkv", bufs=2)
        lo_pool = P(name="lo", bufs=2)
        s_pool = P(name="s", bufs=2)
        p_pool = P(name="p", bufs=2)
        pt_pool = P(name="pt", bufs=2)
        ot_pool = P(name="ot", bufs=3)
        tmp_pool = P(name="tmp", bufs=2)
        stats = P(name="stats", bufs=4)
        ps = P(name="ps", bufs=1, space="PSUM")  # bufs set per tile() call

        def load_x_tg(t0):
            """Create + DMA the X tiles for one 512-token group."""
            xh_t, x8_t = [], []
            # all xh6 DMAs first: the fp16 main chains and V are the first
            # consumers, the fp8 pair tiles are only needed by the later DR
            # chains, so don't let xl8 transfers queue ahead of xh6
            for dc in range(DC):
                th = x_pool.tile([128, 512], f16, tag="xh", name="xh_t")
                nc.sync.dma_start(
                    th[:], xh6_ap[dc * 128 : (dc + 1) * 128, t0 : t0 + 512]
                )
                xh_t.append(th)
            for dc in range(DC):
                t8 = x_pool.tile([128, 2, 2, 256], f8, tag="x8", name="x8_t", bufs=X8_BUFS)
                # slot0 (xh8) derived on-device from xh6 — saves HBM traffic
                nc.vector.tensor_scalar_mul(t8[:, 0, :, :], xh_t[dc][:], 2.0 ** -6)
                nc.sync.dma_start(
                    t8[:, 1, :, :],
                    xl8_ap[dc * 128 : (dc + 1) * 128, t0 : t0 + 512],
                )
                x8_t.append(t8)
            return xh_t, x8_t

        # resident weights
        # fp16 mains: [128, DC*RW], column block dc holds W[dc*128:(dc+1)*128, :]
        # (wq6 first, then batch-0 X prefetch, then the rest — so the first
        # projection chain's operands land early and the PE starts ~15us in)
        w6_sb = {
            m: const.tile([128, DC * RW], f16, tag=f"w{m}6", name=f"w{m}6_sb")
            for m in ("q", "k")
        }
        # fp8 pair weights: [128, 2slot, DC, HPC, 128]; slot0=wl*2^12, slot1=wh
        w8_sb = {
            m: const.tile(
                [128, 2, DC, HPC, 128], f8, tag=f"w{m}8", name=f"w{m}8_sb"
            )
            for m in ("q", "k")
        }

        def load_w(m):
            for dc in range(DC):
                nc.sync.dma_start(
                    w6_sb[m][:, dc * RW : (dc + 1) * RW],
                    w6_ap[m][dc * 128 : (dc + 1) * 128, :],
                )
            for dc in range(DC):
                nc.sync.dma_start(
                    w8_sb[m][:, :, dc, :, :],
                    w8_ap[m][dc * 128 : (dc + 1) * 128, :],
                )

        # DMA issue order follows first use: K chains run first (wk + tg0 X),
        # then V, then Q chains, then the next token group
        load_w("k")
        pre_x = {0: load_x_tg(0)}
        wv_sb = const.tile([128, DC * RW], f16, tag="wv", name="wv_sb")
        for dc in range(DC):
            nc.sync.dma_start(
                wv_sb[:, dc * RW : (dc + 1) * RW],
                wv_ap[dc * 128 : (dc + 1) * 128, :],
            )
        load_w("q")
        pre_x[1] = load_x_tg(512)
        wo_sb = const.tile([128, HPC * R], f16, tag="wo", name="wo_sb")
        for rh in range(HPC):
            nc.sync.dma_start(
                wo_sb[:, rh * R : (rh + 1) * R],
                wo_ap[rh * 128 : (rh + 1) * 128, :],
            )
        ident = const.tile([128, 128], f16, tag="ident", name="ident")
        make_identity(nc, ident[:])

        a_state = {}

        def gen_phase_a(b):
            """Projections for batch b, yielding after each matmul chain (32
            yields) so the caller can interleave them with the previous
            batch's attention iterations."""
            tb0 = b * S
            # hi fp16 tiles: Q^T at 2^4, K^T at 2^5 — [128 rank, S tokens]
            hi = {
                (m, rh): qkv_pool.tile(
                    [128, S], f16, tag=f"{m}h{rh}", name=f"{m}h{rh}"
                )
                for m in ("q", "k")
                for rh in range(HPC)
            }
            # fp8 pair tiles:
            #  q: [128, 2slot, 16qb, 128]  slot0=qh*2^-1, slot1=ql*2^10
            #  k: [128, 2slot, 8chunk, 256] slot0=kl*2^10, slot1=kh*2^-1
            q8t = {
                rh: qkv_pool.tile(
                    [128, 2, 16, 128], f8, tag=f"q8{rh}", name=f"q8{rh}"
                )
                for rh in range(HPC)
            }
            k8t = {
                rh: qkv_pool.tile(
                    [128, 2, 8, 256], f8, tag=f"k8{rh}", name=f"k8{rh}"
                )
                for rh in range(HPC)
            }
            v_sb = qkv_pool.tile([128, DC * RW], f16, tag="v", name="v_sb")
            a_state[b] = (hi, q8t, k8t, v_sb)

            for tg in range(4):
                t0 = tb0 + tg * 512
                if b == 0 and tg in pre_x:
                    xh_t, x8_t = pre_x.pop(tg)
                else:
                    xh_t, x8_t = load_x_tg(t0)

                # Q^T, K^T: fp16 hi*hi chain + fp8 DoubleRow cross chain
                for m in ("k",):
                    for rh in range(HPC):
                        psp = ps.tile([128, 512], f32, tag="pa", bufs=2, name="ps_proj")
                        for dc in range(DC):
                            nc.tensor.matmul(
                                psp[:],
                                lhsT=w6_sb[m][
                                    :, dc * RW + rh * 128 : dc * RW + rh * 128 + 128
                                ],
                                rhs=xh_t[dc][:],
                                start=(dc == 0),
                                stop=False,
                            )
                        for dc in range(DC):
                            nc.tensor.matmul(
                                psp[:],
                                lhsT=w8_sb[m][:, :, dc, rh, :],
                                rhs=x8_t[dc][:, :, :, :],
                                perf_mode=DR,
                                start=False,
                                stop=(dc == DC - 1),
                            )
                        # drain: hi fp16 copy, lo via scalar_tensor_tensor,
                        # then two fp8 converts into the pair tiles
                        hs = hi[(m, rh)][:, tg * 512 : (tg + 1) * 512]
                        sc = 2.0 ** -8 if m == "q" else 2.0 ** -7
                        nc.scalar.mul(hs, psp[:], sc)
                        lo = lo_pool.tile([128, 512], f16, tag="lo", name="lo")
                        nc.vector.scalar_tensor_tensor(
                            lo[:], psp[:], sc, hs, MUL, SUB
                        )
                        if m == "q":
                            nc.scalar.mul(
                                q8t[rh][:, 0, tg * 4 : (tg + 1) * 4, :],
                                hs, 2.0 ** -5,
                            )
                            nc.scalar.mul(
                                q8t[rh][:, 1, tg * 4 : (tg + 1) * 4, :],
                                lo[:], 2.0 ** 6,
                            )
                        else:
                            nc.scalar.mul(
                                k8t[rh][:, 1, tg * 2 : (tg + 1) * 2, :],
                                hs, 2.0 ** -6,
                            )
                            nc.scalar.mul(
                                k8t[rh][:, 0, tg * 2 : (tg + 1) * 2, :],
                                lo[:], 2.0 ** 5,
                            )
                        yield

                # V after K: each V matmul needs every xh tile of the
                # group, while the K mains consume them sequentially and
                # can ramp with the arriving DMAs; Q still runs last
                for tb in range(4):
                    psv = ps.tile([128, RW], f32, tag="pa", bufs=2, name="ps_vproj")
                    for dc in range(DC):
                        nc.tensor.matmul(
                            psv[:],
                            lhsT=xh_t[dc][:, tb * 128 : (tb + 1) * 128],
                            rhs=wv_sb[:, dc * RW : (dc + 1) * RW],
                            start=(dc == 0),
                            stop=(dc == DC - 1),
                        )
                    tbi = tg * 4 + tb
                    nc.scalar.copy(v_sb[:, tbi * RW : (tbi + 1) * RW], psv[:])
                    yield

                # Q^T, K^T: fp16 hi*hi chain + fp8 DoubleRow cross chain
                for m in ("q",):
                    for rh in range(HPC):
                        psp = ps.tile([128, 512], f32, tag="pa", bufs=2, name="ps_proj")
                        for dc in range(DC):
                            nc.tensor.matmul(
                                psp[:],
                                lhsT=w6_sb[m][
                                    :, dc * RW + rh * 128 : dc * RW + rh * 128 + 128
                                ],
                                rhs=xh_t[dc][:],
                                start=(dc == 0),
                                stop=False,
                            )
                        for dc in range(DC):
                            nc.tensor.matmul(
                                psp[:],
                                lhsT=w8_sb[m][:, :, dc, rh, :],
                                rhs=x8_t[dc][:, :, :, :],
                                perf_mode=DR,
                                start=False,
                                stop=(dc == DC - 1),
                            )
                        # drain: hi fp16 copy, lo via scalar_tensor_tensor,
                        # then two fp8 converts into the pair tiles
                        hs = hi[(m, rh)][:, tg * 512 : (tg + 1) * 512]
                        sc = 2.0 ** -8 if m == "q" else 2.0 ** -7
                        nc.scalar.mul(hs, psp[:], sc)
                        lo = lo_pool.tile([128, 512], f16, tag="lo", name="lo")
                        nc.vector.scalar_tensor_tensor(
                            lo[:], psp[:], sc, hs, MUL, SUB
                        )
                        if m == "q":
                            nc.scalar.mul(
                                q8t[rh][:, 0, tg * 4 : (tg + 1) * 4, :],
                                hs, 2.0 ** -5,
                            )
                            nc.scalar.mul(
                                q8t[rh][:, 1, tg * 4 : (tg + 1) * 4, :],
                                lo[:], 2.0 ** 6,
                            )
                        else:
                            nc.scalar.mul(
                                k8t[rh][:, 1, tg * 2 : (tg + 1) * 2, :],
                                hs, 2.0 ** -6,
                            )
                            nc.scalar.mul(
                                k8t[rh][:, 0, tg * 2 : (tg + 1) * 2, :],
                                lo[:], 2.0 ** 5,
                            )
                        yield

        for _ in gen_phase_a(0):  # batch 0 projections up front
            pass

        for b in range(B):
            tb0 = b * S
            hi, q8t, k8t, v_sb = a_state.pop(b)
            nxt = gen_phase_a(b + 1) if b + 1 < B else iter(())

            # ── phase B: attention, heads interleaved per q-block, with the
            # next batch's projection chains interleaved one per iteration ──
            for qb in range(16):
                o2s = []
                for h in range(HPC):
                    q0 = qb * 128
                    # scores psum = 2^9 * S: fp16 (qh*2^4)·(kh*2^5) + fp8 DR
                    # (qh*2^-1)·(kl*2^10) + (ql*2^10)·(kh*2^-1)
                    s_sb = s_pool.tile([128, S], f32, tag="s", name="s_sb")
                    pmax = stats.tile([128, 4], f32, tag="pmax", name="pmax")
                    # kt pairs: both fp16 mains back-to-back (one qh4 LDW
                    # region), then 4 DR matmuls sharing the q8 stationary
                    for kp in range(2):
                        pss = [
                            ps.tile([128, 512], f32, tag="s", bufs=3, name="ps_s")
                            for _ in range(2)
                        ]
                        for i in range(2):
                            kt = kp * 2 + i
                            nc.tensor.matmul(
                                pss[i][:],
                                lhsT=hi[("q", h)][:, q0 : q0 + 128],
                                rhs=hi[("k", h)][:, kt * 512 : (kt + 1) * 512],
                                start=True,
                                stop=False,
                            )
                        for i in range(2):
                            kt = kp * 2 + i
                            nc.tensor.matmul(
                                pss[i][:],
                                lhsT=q8t[h][:, :, qb, :],
                                rhs=k8t[h][:, :, kt * 2 : kt * 2 + 2, :],
                                perf_mode=DR,
                                start=False,
                                stop=True,
                            )
                        for i in range(2):
                            kt = kp * 2 + i
                            sl = s_sb[:, kt * 512 : (kt + 1) * 512]
                            # balance Scalar vs Vector: mid-run the Scalar
                            # engine also carries projection drains, so one
                            # copy goes to the DVE; the last batch instead
                            # overloads the DVE (reductions + pt copies), so
                            # all copies stay on Scalar there
                            if kt == 3 and b < B - 1:
                                nc.vector.tensor_copy(sl, pss[i][:])
                            else:
                                nc.scalar.copy(sl, pss[i][:])
                            nc.vector.reduce_max(
                                pmax[:, kt : kt + 1], pss[i][:], axis=AX.X
                            )

                    negmax = stats.tile([128, 1], f32, tag="negmax", name="negmax")
                    nc.vector.reduce_max(negmax[:], pmax[:], axis=AX.X, negate=True)
                    bias = stats.tile([128, 1], f32, tag="bias", name="bias")
                    nc.vector.tensor_scalar_mul(bias[:], negmax[:], SCALE9)
                    p_t = p_pool.tile([128, S], f16, tag="p", name="p_t")
                    pt_sb = pt_pool.tile([128, DC * 128], f16, tag="pt", name="pt_sb")
                    ssum4 = stats.tile([128, 4], f32, tag="ssum4", name="ssum4")
                    # two 1024-wide exps (fewer Scalar dispatches); the
                    # transposes/psum copies still pipeline per 512 slice
                    for ep in range(2):
                        nc.scalar.activation(
                            p_t[:, ep * 1024 : (ep + 1) * 1024],
                            s_sb[:, ep * 1024 : (ep + 1) * 1024],
                            EXP, bias=bias[:], scale=SCALE9,
                            accum_out=ssum4[:, ep : ep + 1],
                        )
                        for kt in (ep * 2, ep * 2 + 1):
                            pst = ps.tile(
                                [128, 512], f16, tag="pt", bufs=2, name="ps_pt"
                            )
                            for j in range(4):
                                kc = kt * 4 + j
                                nc.tensor.transpose(
                                    pst[:, j * 128 : (j + 1) * 128],
                                    p_t[:, kc * 128 : (kc + 1) * 128],
                                    ident[:],
                                )
                            nc.vector.tensor_copy(
                                pt_sb[:, kt * 512 : (kt + 1) * 512], pst[:]
                            )
                    ssum = stats.tile([128, 1], f32, tag="ssum", name="ssum")
                    nc.vector.reduce_sum(ssum[:], ssum4[:, 0:2], axis=AX.X)
                    rc = stats.tile([128, 1], f32, tag=f"recip{h}", name="rc")
                    nc.vector.reciprocal(rc[:], ssum[:])

                    # attn = P @ V, accumulated transposed: O^T [128 r, 128 q]
                    ps_ot = ps.tile([128, 128], f32, tag="ot", bufs=1, name="ps_ot")
                    for kc in range(DC):
                        nc.tensor.matmul(
                            ps_ot[:],
                            lhsT=v_sb[
                                :, kc * RW + h * 128 : kc * RW + h * 128 + 128
                            ],
                            rhs=pt_sb[:, kc * 128 : (kc + 1) * 128],
                            start=(kc == 0),
                            stop=(kc == DC - 1),
                        )
                    ot_sb = ot_pool.tile([128, 128], f16, tag="ot", name="ot_sb")
                    if b == B - 1:
                        nc.vector.tensor_copy(ot_sb[:], ps_ot[:])
                    else:
                        nc.scalar.copy(ot_sb[:], ps_ot[:])

                    # out2 [128 q, 128] = O^T.T @ Wo_h  (fp16)
                    ps_o2 = ps.tile([128, 512], f32, tag="pa", bufs=2, name="ps_o2")
                    nc.tensor.matmul(
                        ps_o2[:, 0:128],
                        lhsT=ot_sb[:],
                        rhs=wo_sb[:, h * R : (h + 1) * R],
                        start=True,
                        stop=True,
                    )
                    tmp = tmp_pool.tile([128, 128], f32, tag=f"o2s{h}", name="tmp")
                    if b == B - 1:
                        nc.vector.tensor_scalar_mul(tmp[:], ps_o2[:, 0:128], rc[:])
                    else:
                        nc.scalar.mul(tmp[:], ps_o2[:, 0:128], rc[:])
                    o2s.append(tmp)
                    next(nxt, None)  # interleave one next-batch proj chain
                res = tmp_pool.tile([128, 128], f32, tag="res", name="res")
                nc.vector.tensor_add(res[:], o2s[0][:], o2s[1][:])
                nc.sync.dma_start(
                    ar_in.ap()[tb0 + qb * 128 : tb0 + (qb + 1) * 128, :],
                    res[:],
                )
                if b == B - 1:
                    done = (qb + 1) * 128
                    off = 0
                    for r0, nr in TAIL_CHUNKS:
                        if r0 + nr == done:
                            o0 = b * 256 + off
                            on = nr // N_CORES
                            nc.gpsimd.collective_compute(
                                "ReduceScatter",
                                mybir.AluOpType.add,
                                replica_groups=[list(range(N_CORES))],
                                ins=[ar_in.ap()[tb0 + r0 : tb0 + r0 + nr, :]],
                                outs=[rs_out.ap()[o0 : o0 + on, :]],
                            )
                            nc.sync.dma_start(
                                out_ap[o0 : o0 + on, :],
                                rs_out.ap()[o0 : o0 + on, :],
                            )
                        off += nr // N_CORES

            for _ in nxt:  # drain any leftover projection chains
                pass
            # reduce-scatter this batch's slice while the next batch computes
            if b < B - 1:
                nc.gpsimd.collective_compute(
                    "ReduceScatter",
                    mybir.AluOpType.add,
                    replica_groups=[list(range(N_CORES))],
                    ins=[ar_in.ap()[tb0 : tb0 + S, :]],
                    outs=[rs_out.ap()[b * 256 : (b + 1) * 256, :]],
                )
                nc.sync.dma_start(
                    out_ap[b * 256 : (b + 1) * 256, :],
                    rs_out.ap()[b * 256 : (b + 1) * 256, :],
                )

    return nc


# ─────────────────────────────── host entry ───────────────────────────────
def _f8(a):
    return np.asarray(a, np.float32).astype(F8NP)


def kernel(X, mask, W_Q, W_K, W_V, W_O):
    _install_ntff_hook()
    from concourse.bass_utils import run_bass_kernel_spmd

    X2 = np.ascontiguousarray(
        np.asarray(X, dtype=np.float32).reshape(T, D).T
    )  # [D, T]
    xh32 = X2.astype(np.float16).astype(np.float32)
    xl32 = X2 - xh32
    xh6 = (xh32 * 64.0).astype(np.float16)
    xl8 = _f8(xl32 * 4096.0)
    W_Q = np.asarray(W_Q, np.float32)
    W_K = np.asarray(W_K, np.float32)
    W_V = np.asarray(W_V, np.float32)
    W_O = np.asarray(W_O, np.float32)

    in_maps = []
    for c in range(N_CORES):
        cols = slice(c * RW, (c + 1) * RW)

        def wsplit(Wfull):
            wc = Wfull[:, cols]
            wh32 = wc.astype(np.float16).astype(np.float32)
            wl32 = wc - wh32
            w6 = (wh32 * 64.0).astype(np.float16)
            w8 = np.empty((D, 2, RW), F8NP)
            w8[:, 0, :] = _f8(wl32 * 4096.0)
            w8[:, 1, :] = _f8(wh32)
            return w6, w8.reshape(D, 2 * RW)

        wq6, wq8 = wsplit(W_Q)
        wk6, wk8 = wsplit(W_K)
        wv6m = (
            W_V[:, cols].astype(np.float16).astype(np.float32) * 2.0 ** -6
        ).astype(np.float16)
        in_maps.append(
            {
                "xh6": xh6,
                "xl8": xl8,
                "wq6": wq6,
                "wq8": wq8,
                "wk6": wk6,
                "wk8": wk8,
                "wv6m": wv6m,
                "wo": np.ascontiguousarray(W_O[cols, :]).astype(np.float16),
            }
        )

    nc = _build_nc()
    _split_excess_waits(nc)
    trace = bool(int(os.environ.get("KERNEL_TRACE", "0")))
    res = run_bass_kernel_spmd(
        nc, in_maps, list(range(N_CORES)), trace=trace
    )
    LAST_EXEC_TIME_NS[0] = res.exec_time_ns
    LAST_RESULTS[0] = res
    # reassemble the reduce-scattered output: batches 0..B-2 were scattered
    # as whole [S, R] chunks (core c holds rows c*256..), the last batch as
    # four 512-row chunks (core c holds rows j*512 + c*64..)
    full = np.empty((T, R), np.float32)
    for c in range(N_CORES):
        oc = np.asarray(res.results[c]["out"], dtype=np.float32)
        for b in range(B - 1):
            full[b * S + c * 256 : b * S + (c + 1) * 256] = oc[
                b * 256 : (b + 1) * 256
            ]
        b = B - 1
        off = 0
        for r0, nr in TAIL_CHUNKS:
            on = nr // N_CORES
            full[b * S + r0 + c * on : b * S + r0 + (c + 1) * on] = oc[
                b * 256 + off : b * 256 + off + on
            ]
            off += on
    return full.reshape(B, S, R)


# revision 61
# speedup vs baseline: 1.0176x; 1.0159x over previous
"""Multi-head attention (B=4, S=2048, MODEL_DIM=2048, 16 heads, head dim 128)
on 8 Trainium2 NeuronCores.

Sharding: tensor-parallel over heads — 2 heads per core.  Each core projects
all 8192 tokens through its 256-column slice of W_Q/W_K/W_V, runs attention
for its heads, applies its 256-row slice of W_O, and a per-batch
ReduceScatter sums the partial outputs (overlapping compute; each core keeps
its 1/8 row-chunk and the host reassembles the full output).  The last
batch's reductions fire per row-chunk as q-blocks finish so only the final
small chunk is exposed at the tail.

Numerics: the softmax path is precision-critical (scores have std ~2048, so
the softmax is near-argmax; small score errors flip near-tie rows).  The Q/K
projections and the Q.K^T scores run as a 2-pass scheme: one fp16 hi*hi
matmul pass (exact products, f32 accum) plus ONE fp8 DoubleRow pass that
computes both cross terms (hi*lo + lo*hi) at 2 MACs/cell/cycle.  Hi/lo limbs
carry power-of-2 scales chosen so the fp16 pass and the fp8 pass accumulate
into the same PSUM bank at a common scale (projection psum = 2^12 * Q,
score psum = 2^9 * S), so no extra combine ops are needed.  The value path
runs single-pass fp16 (V projection, P.V, W_O) with exact f32 softmax
statistics throughout.
"""

import os
import sys
import types

sys.path.insert(0, "/opt/trn_rl_repo")

import numpy as np
import ml_dtypes

# ─────────────────────────────── constants ───────────────────────────────
B, S, D = 4, 2048, 2048
H, R = 16, 128
N_CORES = 8
HPC = H // N_CORES          # heads per core = 2
RW = HPC * R                # per-core projection width = 256
T = B * S                   # 8192 tokens
DC = D // 128               # 16 contraction chunks
SCALE = 1.0 / (R ** 0.5)
SCALE9 = SCALE / 512.0      # score psum carries a 2^9 scale

F8NP = ml_dtypes.float8_e4m3   # TRN float8e4 (max +-240)

XH_BUFS = int(os.environ.get("K_XH_BUFS", "28"))
X8_BUFS = int(os.environ.get("K_X8_BUFS", "20"))

# last-batch reduce-scatter chunks: (start row in batch, nrows), fired as
# soon as the covering q-blocks finish; the final small chunk minimizes the
# exposed tail
TAIL_CHUNKS = [(0, 1536), (1536, 512)]

LAST_EXEC_TIME_NS = [None]
LAST_RESULTS = [None]


# ───────────────────────── harness glue (inlined) ─────────────────────────
def _install_ntff_hook():
    """Wire the missing antenv.axon_hooks module so trace=True can profile."""
    try:
        import antenv.axon_hooks  # noqa: F401
        return
    except ImportError:
        pass
    try:
        import antenv
        from trn_agent_boot.trn_boot import _ntff_profile_via_ctypes
    except ImportError:
        return
    mod = types.ModuleType("antenv.axon_hooks")
    _hook = [None]
    mod.set_axon_ntff_profile_hook = lambda h: _hook.__setitem__(0, h)
    mod.get_axon_ntff_profile_hook = lambda: _hook[0]
    antenv.axon_hooks = mod
    sys.modules["antenv.axon_hooks"] = mod
    try:
        mod.set_axon_ntff_profile_hook(
            _ntff_profile_via_ctypes("/opt/axon/libaxon_pjrt.so")
        )
    except Exception:
        pass


def _split_excess_waits(nc, max_waits=1):
    """walrus on this toolchain rejects >1 sem-wait per instruction; hoist
    the excess onto preceding same-engine NoOps."""
    from concourse import mybir

    for fn in nc.m.functions:
        for bb in fn.blocks:
            insts = list(bb.instructions)
            out = []
            changed = False
            for inst in insts:
                si = inst.sync_info
                if si is not None and si.on_wait and len(si.on_wait) > max_waits:
                    waits = list(si.on_wait)
                    chunks = [
                        waits[i : i + max_waits]
                        for i in range(0, len(waits), max_waits)
                    ]
                    for ci, chunk in enumerate(chunks[:-1]):
                        out.append(
                            mybir.InstNoOp(
                                name=f"{inst.name}-ws{ci}",
                                engine=inst.engine,
                                ins=[],
                                outs=[],
                                sync_info=mybir.SyncInfo(
                                    on_wait=list(chunk), on_update=[]
                                ),
                                text_hint="waitsplit",
                            )
                        )
                    si.on_wait = list(chunks[-1])
                    changed = True
                out.append(inst)
            if changed:
                try:
                    bb.instructions = out
                except Exception:
                    bb.instructions.clear()
                    for i in out:
                        bb.instructions.append(i)


# ───────────────────────────── device kernel ─────────────────────────────
def _build_nc():
    from contextlib import ExitStack

    import concourse.bass as bass
    import concourse.tile as tile
    from concourse import mybir
    from concourse.masks import make_identity

    f32 = mybir.dt.float32
    f16 = mybir.dt.float16
    bf16 = mybir.dt.bfloat16
    f8 = mybir.dt.float8e4
    AX = mybir.AxisListType
    EXP = mybir.ActivationFunctionType.Exp
    DR = mybir.MatmulPerfMode.DoubleRow
    MUL = mybir.AluOpType.mult
    SUB = mybir.AluOpType.subtract
    MAX = mybir.AluOpType.max

    nc = bass.Bass(
        "TRN2", target_bir_lowering=False, debug=False, num_devices=N_CORES
    )

    xh6_ap = nc.dram_tensor("xh6", [D, T], f16, kind="ExternalInput").ap()
    xl8_ap = nc.dram_tensor("xl8", [D, T], f8, kind="ExternalInput").ap()
    w6_ap = {
        m: nc.dram_tensor(f"w{m}6", [D, RW], f16, kind="ExternalInput").ap()
        for m in ("q", "k")
    }
    w8_ap = {
        m: nc.dram_tensor(f"w{m}8", [D, 2 * RW], f8, kind="ExternalInput").ap()
        for m in ("q", "k")
    }
    wv_ap = nc.dram_tensor("wv6m", [D, RW], f16, kind="ExternalInput").ap()
    wo_ap = nc.dram_tensor("wo", [RW, R], f16, kind="ExternalInput").ap()
    # ReduceScatter output: each core keeps its contiguous 1/8 of every
    # reduced row-chunk; the host reassembles the full [T, R].
    out_ap = nc.dram_tensor(
        "out", [B * (S // N_CORES), R], f32, kind="ExternalOutput"
    ).ap()
    ar_in = nc.dram_tensor("ar_in", [T, R], f32)
    rs_out = nc.dram_tensor("rs_out", [B * (S // N_CORES), R], f32)

    with tile.TileContext(nc) as tc, ExitStack() as ctx:
        P = lambda **kw: ctx.enter_context(tc.tile_pool(**kw))
        const = P(name="const", bufs=1)
        x_pool = P(name="x", bufs=XH_BUFS)
        qkv_pool = P(name="qkv", bufs=2)
        lo_pool = P(name="lo", bufs=2)
        s_pool = P(name="s", bufs=2)
        p_pool = P(name="p", bufs=2)
        pt_pool = P(name="pt", bufs=2)
        ot_pool = P(name="ot", bufs=3)
        tmp_pool = P(name="tmp", bufs=2)
        stats = P(name="stats", bufs=4)
        ps = P(name="ps", bufs=1, space="PSUM")  # bufs set per tile() call

        def load_x_tg(t0):
            """Create + DMA the X tiles for one 512-token group."""
            xh_t, x8_t = [], []
            # all xh6 DMAs first: the fp16 main chains and V are the first
            # consumers, the fp8 pair tiles are only needed by the later DR
            # chains, so don't let xl8 transfers queue ahead of xh6
            for dc in range(DC):
                th = x_pool.tile([128, 512], f16, tag="xh", name="xh_t")
                nc.sync.dma_start(
                    th[:], xh6_ap[dc * 128 : (dc + 1) * 128, t0 : t0 + 512]
                )
                xh_t.append(th)
            for dc in range(DC):
                t8 = x_pool.tile([128, 2, 2, 256], f8, tag="x8", name="x8_t", bufs=X8_BUFS)
                # slot0 (xh8) derived on-device from xh6 — saves HBM traffic
                nc.vector.tensor_scalar_mul(t8[:, 0, :, :], xh_t[dc][:], 2.0 ** -6)
                nc.sync.dma_start(
                    t8[:, 1, :, :],
                    xl8_ap[dc * 128 : (dc + 1) * 128, t0 : t0 + 512],
                )
                x8_t.append(t8)
            return xh_t, x8_t

        # resident weights
        # fp16 mains: [128, DC*RW], column block dc holds W[dc*128:(dc+1)*128, :]
        # (wq6 first, then batch-0 X prefetch, then the rest — so the first
        # projection chain's operands land early and the PE starts ~15us in)
        w6_sb = {
            m: const.tile([128, DC * RW], f16, tag=f"w{m}6", name=f"w{m}6_sb")
            for m in ("q", "k")
        }
        # fp8 pair weights: [128, 2slot, DC, HPC, 128]; slot0=wl*2^12, slot1=wh
        w8_sb = {
            m: const.tile(
                [128, 2, DC, HPC, 128], f8, tag=f"w{m}8", name=f"w{m}8_sb"
            )
            for m in ("q", "k")
        }

        def load_w(m):
            for dc in range(DC):
                nc.sync.dma_start(
                    w6_sb[m][:, dc * RW : (dc + 1) * RW],
                    w6_ap[m][dc * 128 : (dc + 1) * 128, :],
                )
            for dc in range(DC):
                nc.sync.dma_start(
                    w8_sb[m][:, :, dc, :, :],
                    w8_ap[m][dc * 128 : (dc + 1) * 128, :],
                )

        # DMA issue order follows first use: V chains run first (wv + tg0 X),
        # then K chains, then Q chains, then the next token group
        wv_sb = const.tile([128, DC * RW], f16, tag="wv", name="wv_sb")
        for dc in range(DC):
            nc.sync.dma_start(
                wv_sb[:, dc * RW : (dc + 1) * RW],
                wv_ap[dc * 128 : (dc + 1) * 128, :],
            )
        pre_x = {0: load_x_tg(0)}
        load_w("k")
        load_w("q")
        pre_x[1] = load_x_tg(512)
        wo_sb = const.tile([128, HPC * R], f16, tag="wo", name="wo_sb")
        for rh in range(HPC):
            nc.sync.dma_start(
                wo_sb[:, rh * R : (rh + 1) * R],
                wo_ap[rh * 128 : (rh + 1) * 128, :],
            )
        ident = const.tile([128, 128], f16, tag="ident", name="ident")
        make_identity(nc, ident[:])

        a_state = {}

        def gen_phase_a(b):
            """Projections for batch b, yielding after each matmul chain (32
            yields) so the caller can interleave them with the previous
            batch's attention iterations."""
            tb0 = b * S
            # hi fp16 tiles: Q^T at 2^4, K^T at 2^5 — [128 rank, S tokens]
            hi = {
                (m, rh): qkv_pool.tile(
                    [128, S], f16, tag=f"{m}h{rh}", name=f"{m}h{rh}"
                )
                for m in ("q", "k")
                for rh in range(HPC)
            }
            # fp8 pair tiles:
            #  q: [128, 2slot, 16qb, 128]  slot0=qh*2^-1, slot1=ql*2^10
            #  k: [128, 2slot, 8chunk, 256] slot0=kl*2^10, slot1=kh*2^-1
            q8t = {
                rh: qkv_pool.tile(
                    [128, 2, 16, 128], f8, tag=f"q8{rh}", name=f"q8{rh}"
                )
                for rh in range(HPC)
            }
            k8t = {
                rh: qkv_pool.tile(
                    [128, 2, 8, 256], f8, tag=f"k8{rh}", name=f"k8{rh}"
                )
                for rh in range(HPC)
            }
            v_sb = qkv_pool.tile([128, DC * RW], f16, tag="v", name="v_sb")
            a_state[b] = (hi, q8t, k8t, v_sb)

            for tg in range(4):
                t0 = tb0 + tg * 512
                if b == 0 and tg in pre_x:
                    xh_t, x8_t = pre_x.pop(tg)
                else:
                    xh_t, x8_t = load_x_tg(t0)

                # V first (natural layout [t, r]); then K chains before Q so
                # the next batch's attention unblocks as early as possible
                for tb in range(4):
                    psv = ps.tile([128, RW], f32, tag="pa", bufs=2, name="ps_vproj")
                    for dc in range(DC):
                        nc.tensor.matmul(
                            psv[:],
                            lhsT=xh_t[dc][:, tb * 128 : (tb + 1) * 128],
                            rhs=wv_sb[:, dc * RW : (dc + 1) * RW],
                            start=(dc == 0),
                            stop=(dc == DC - 1),
                        )
                    tbi = tg * 4 + tb
                    nc.scalar.copy(v_sb[:, tbi * RW : (tbi + 1) * RW], psv[:])
                    yield

                # Q^T, K^T: fp16 hi*hi chain + fp8 DoubleRow cross chain
                for m in ("k", "q"):
                    for rh in range(HPC):
                        psp = ps.tile([128, 512], f32, tag="pa", bufs=2, name="ps_proj")
                        for dc in range(DC):
                            nc.tensor.matmul(
                                psp[:],
                                lhsT=w6_sb[m][
                                    :, dc * RW + rh * 128 : dc * RW + rh * 128 + 128
                                ],
                                rhs=xh_t[dc][:],
                                start=(dc == 0),
                                stop=False,
                            )
                        for dc in range(DC):
                            nc.tensor.matmul(
                                psp[:],
                                lhsT=w8_sb[m][:, :, dc, rh, :],
                                rhs=x8_t[dc][:, :, :, :],
                                perf_mode=DR,
                                start=False,
                                stop=(dc == DC - 1),
                            )
                        # drain: hi fp16 copy, lo via scalar_tensor_tensor,
                        # then two fp8 converts into the pair tiles
                        hs = hi[(m, rh)][:, tg * 512 : (tg + 1) * 512]
                        sc = 2.0 ** -8 if m == "q" else 2.0 ** -7
                        nc.scalar.mul(hs, psp[:], sc)
                        lo = lo_pool.tile([128, 512], f16, tag="lo", name="lo")
                        nc.vector.scalar_tensor_tensor(
                            lo[:], psp[:], sc, hs, MUL, SUB
                        )
                        if m == "q":
                            nc.scalar.mul(
                                q8t[rh][:, 0, tg * 4 : (tg + 1) * 4, :],
                                hs, 2.0 ** -5,
                            )
                            nc.scalar.mul(
                                q8t[rh][:, 1, tg * 4 : (tg + 1) * 4, :],
                                lo[:], 2.0 ** 6,
                            )
                        else:
                            nc.scalar.mul(
                                k8t[rh][:, 1, tg * 2 : (tg + 1) * 2, :],
                                hs, 2.0 ** -6,
                            )
                            nc.scalar.mul(
                                k8t[rh][:, 0, tg * 2 : (tg + 1) * 2, :],
                                lo[:], 2.0 ** 5,
                            )
                        yield

        for _ in gen_phase_a(0):  # batch 0 projections up front
            pass

        for b in range(B):
            tb0 = b * S
            hi, q8t, k8t, v_sb = a_state.pop(b)
            nxt = gen_phase_a(b + 1) if b + 1 < B else iter(())

            # ── phase B: attention, heads interleaved per q-block, with the
            # next batch's projection chains interleaved one per iteration ──
            for qb in range(16):
                o2s = []
                for h in range(HPC):
                    q0 = qb * 128
                    # scores psum = 2^9 * S: fp16 (qh*2^4)·(kh*2^5) + fp8 DR
                    # (qh*2^-1)·(kl*2^10) + (ql*2^10)·(kh*2^-1)
                    s_sb = s_pool.tile([128, S], f32, tag="s", name="s_sb")
                    pmax = stats.tile([128, 4], f32, tag="pmax", name="pmax")
                    # kt pairs: both fp16 mains back-to-back (one qh4 LDW
                    # region), then 4 DR matmuls sharing the q8 stationary
                    for kp in range(2):
                        pss = [
                            ps.tile([128, 512], f32, tag="s", bufs=3, name="ps_s")
                            for _ in range(2)
                        ]
                        for i in range(2):
                            kt = kp * 2 + i
                            nc.tensor.matmul(
                                pss[i][:],
                                lhsT=hi[("q", h)][:, q0 : q0 + 128],
                                rhs=hi[("k", h)][:, kt * 512 : (kt + 1) * 512],
                                start=True,
                                stop=False,
                            )
                        for i in range(2):
                            kt = kp * 2 + i
                            nc.tensor.matmul(
                                pss[i][:],
                                lhsT=q8t[h][:, :, qb, :],
                                rhs=k8t[h][:, :, kt * 2 : kt * 2 + 2, :],
                                perf_mode=DR,
                                start=False,
                                stop=True,
                            )
                        for i in range(2):
                            kt = kp * 2 + i
                            sl = s_sb[:, kt * 512 : (kt + 1) * 512]
                            # balance Scalar vs Vector: mid-run the Scalar
                            # engine also carries projection drains, so one
                            # copy goes to the DVE; the last batch instead
                            # overloads the DVE (reductions + pt copies), so
                            # all copies stay on Scalar there
                            if kt == 3 and b < B - 1:
                                nc.vector.tensor_copy(sl, pss[i][:])
                            else:
                                nc.scalar.copy(sl, pss[i][:])
                            nc.vector.reduce_max(
                                pmax[:, kt : kt + 1], pss[i][:], axis=AX.X
                            )

                    negmax = stats.tile([128, 1], f32, tag="negmax", name="negmax")
                    nc.vector.reduce_max(negmax[:], pmax[:], axis=AX.X, negate=True)
                    bias = stats.tile([128, 1], f32, tag="bias", name="bias")
                    nc.vector.tensor_scalar_mul(bias[:], negmax[:], SCALE9)
                    p_t = p_pool.tile([128, S], f16, tag="p", name="p_t")
                    pt_sb = pt_pool.tile([128, DC * 128], f16, tag="pt", name="pt_sb")
                    ssum4 = stats.tile([128, 4], f32, tag="ssum4", name="ssum4")
                    # two 1024-wide exps (fewer Scalar dispatches); the
                    # transposes/psum copies still pipeline per 512 slice
                    for ep in range(2):
                        nc.scalar.activation(
                            p_t[:, ep * 1024 : (ep + 1) * 1024],
                            s_sb[:, ep * 1024 : (ep + 1) * 1024],
                            EXP, bias=bias[:], scale=SCALE9,
                            accum_out=ssum4[:, ep : ep + 1],
                        )
                        for kt in (ep * 2, ep * 2 + 1):
                            pst = ps.tile(
                                [128, 512], f16, tag="pt", bufs=2, name="ps_pt"
                            )
                            for j in range(4):
                                kc = kt * 4 + j
                                nc.tensor.transpose(
                                    pst[:, j * 128 : (j + 1) * 128],
                                    p_t[:, kc * 128 : (kc + 1) * 128],
                                    ident[:],
                                )
                            nc.vector.tensor_copy(
                                pt_sb[:, kt * 512 : (kt + 1) * 512], pst[:]
                            )
                    ssum = stats.tile([128, 1], f32, tag="ssum", name="ssum")
                    nc.vector.reduce_sum(ssum[:], ssum4[:, 0:2], axis=AX.X)
                    rc = stats.tile([128, 1], f32, tag=f"recip{h}", name="rc")
                    nc.vector.reciprocal(rc[:], ssum[:])

                    # attn = P @ V, accumulated transposed: O^T [128 r, 128 q]
                    ps_ot = ps.tile([128, 128], f32, tag="ot", bufs=1, name="ps_ot")
                    for kc in range(DC):
                        nc.tensor.matmul(
                            ps_ot[:],
                            lhsT=v_sb[
                                :, kc * RW + h * 128 : kc * RW + h * 128 + 128
                            ],
                            rhs=pt_sb[:, kc * 128 : (kc + 1) * 128],
                            start=(kc == 0),
                            stop=(kc == DC - 1),
                        )
                    ot_sb = ot_pool.tile([128, 128], f16, tag="ot", name="ot_sb")
                    if b == B - 1:
                        nc.vector.tensor_copy(ot_sb[:], ps_ot[:])
                    else:
                        nc.scalar.copy(ot_sb[:], ps_ot[:])

                    # out2 [128 q, 128] = O^T.T @ Wo_h  (fp16)
                    ps_o2 = ps.tile([128, 512], f32, tag="pa", bufs=2, name="ps_o2")
                    nc.tensor.matmul(
                        ps_o2[:, 0:128],
                        lhsT=ot_sb[:],
                        rhs=wo_sb[:, h * R : (h + 1) * R],
                        start=True,
                        stop=True,
                    )
                    tmp = tmp_pool.tile([128, 128], f32, tag=f"o2s{h}", name="tmp")
                    if b == B - 1:
                        nc.vector.tensor_scalar_mul(tmp[:], ps_o2[:, 0:128], rc[:])
                    else:
                        nc.scalar.mul(tmp[:], ps_o2[:, 0:128], rc[:])
                    o2s.append(tmp)
                    next(nxt, None)  # interleave one next-batch proj chain
                res = tmp_pool.tile([128, 128], f32, tag="res", name="res")
                nc.vector.tensor_add(res[:], o2s[0][:], o2s[1][:])
                nc.sync.dma_start(
                    ar_in.ap()[tb0 + qb * 128 : tb0 + (qb + 1) * 128, :],
                    res[:],
                )
                if b == B - 1:
                    done = (qb + 1) * 128
                    off = 0
                    for r0, nr in TAIL_CHUNKS:
                        if r0 + nr == done:
                            o0 = b * 256 + off
                            on = nr // N_CORES
                            nc.gpsimd.collective_compute(
                                "ReduceScatter",
                                mybir.AluOpType.add,
                                replica_groups=[list(range(N_CORES))],
                                ins=[ar_in.ap()[tb0 + r0 : tb0 + r0 + nr, :]],
                                outs=[rs_out.ap()[o0 : o0 + on, :]],
                            )
                            nc.sync.dma_start(
                                out_ap[o0 : o0 + on, :],
                                rs_out.ap()[o0 : o0 + on, :],
                            )
                        off += nr // N_CORES

            for _ in nxt:  # drain any leftover projection chains
                pass
            # reduce-scatter this batch's slice while the next batch computes
            if b < B - 1:
                nc.gpsimd.collective_compute(
                    "ReduceScatter",
                    mybir.AluOpType.add,
                    replica_groups=[list(range(N_CORES))],
                    ins=[ar_in.ap()[tb0 : tb0 + S, :]],
                    outs=[rs_out.ap()[b * 256 : (b + 1) * 256, :]],
                )
                nc.sync.dma_start(
                    out_ap[b * 256 : (b + 1) * 256, :],
                    rs_out.ap()[b * 256 : (b + 1) * 256, :],
                )

    return nc


# ─────────────────────────────── host entry ───────────────────────────────
def _f8(a):
    return np.asarray(a, np.float32).astype(F8NP)


def kernel(X, mask, W_Q, W_K, W_V, W_O):
    _install_ntff_hook()
    from concourse.bass_utils import run_bass_kernel_spmd

    X2 = np.ascontiguousarray(
        np.asarray(X, dtype=np.float32).reshape(T, D).T
    )  # [D, T]
    xh32 = X2.astype(np.float16).astype(np.float32)
    xl32 = X2 - xh32
    xh6 = (xh32 * 64.0).astype(np.float16)
    xl8 = _f8(xl32 * 4096.0)
    W_Q = np.asarray(W_Q, np.float32)
    W_K = np.asarray(W_K, np.float32)
    W_V = np.asarray(W_V, np.float32)
    W_O = np.asarray(W_O, np.float32)

    in_maps = []
    for c in range(N_CORES):
        cols = slice(c * RW, (c + 1) * RW)

        def wsplit(Wfull):
            wc = Wfull[:, cols]
            wh32 = wc.astype(np.float16).astype(np.float32)
            wl32 = wc - wh32
            w6 = (wh32 * 64.0).astype(np.float16)
            w8 = np.empty((D, 2, RW), F8NP)
            w8[:, 0, :] = _f8(wl32 * 4096.0)
            w8[:, 1, :] = _f8(wh32)
            return w6, w8.reshape(D, 2 * RW)

        wq6, wq8 = wsplit(W_Q)
        wk6, wk8 = wsplit(W_K)
        wv6m = (
            W_V[:, cols].astype(np.float16).astype(np.float32) * 2.0 ** -6
        ).astype(np.float16)
        in_maps.append(
            {
                "xh6": xh6,
                "xl8": xl8,
                "wq6": wq6,
                "wq8": wq8,
                "wk6": wk6,
                "wk8": wk8,
                "wv6m": wv6m,
                "wo": np.ascontiguousarray(W_O[cols, :]).astype(np.float16),
            }
        )

    nc = _build_nc()
    _split_excess_waits(nc)
    trace = bool(int(os.environ.get("KERNEL_TRACE", "0")))
    res = run_bass_kernel_spmd(
        nc, in_maps, list(range(N_CORES)), trace=trace
    )
    LAST_EXEC_TIME_NS[0] = res.exec_time_ns
    LAST_RESULTS[0] = res
    # reassemble the reduce-scattered output: batches 0..B-2 were scattered
    # as whole [S, R] chunks (core c holds rows c*256..), the last batch as
    # four 512-row chunks (core c holds rows j*512 + c*64..)
    full = np.empty((T, R), np.float32)
    for c in range(N_CORES):
        oc = np.asarray(res.results[c]["out"], dtype=np.float32)
        for b in range(B - 1):
            full[b * S + c * 256 : b * S + (c + 1) * 256] = oc[
                b * 256 : (b + 1) * 256
            ]
        b = B - 1
        off = 0
        for r0, nr in TAIL_CHUNKS:
            on = nr // N_CORES
            full[b * S + r0 + c * on : b * S + r0 + (c + 1) * on] = oc[
                b * 256 + off : b * 256 + off + on
            ]
            off += on
    return full.reshape(B, S, R)
